# revision 41
# baseline (speedup 1.0000x reference)
"""Trainium2 Bass kernel for nn_KmerEmbed: conv1d(one-hot kmer filters) + relu + window-sum.

Computes, for seqs (32,32,30,21), weight (8000,20,3), bias (8000,):
  out[n,m,f] = sum_l relu( sum_{a,j} seqs[n,m,l+j,a(<20)]*weight[f,a,j] + bias[f] )
with l over the 28 valid conv positions; returns (32,32,8000) float32.

Strategy (8 NeuronCores, data-parallel over the 1024 flattened rows, 128 rows/core):
  - im2col on host: the 128 rows x 28 positions = 3584 (nm,l) pairs are packed
    densely into 28 tiles of 128 PSUM partitions (full PE width), K=61 rows
    (60 one-hot taps + bias row).
  - conv = matmul vs the replicated filter matrix Wb (61,8000) f16; tiles are
    packed in pairs into PE row-groups (partitions 0-60 / 64-124) so two
    matmuls stream concurrently; f chunked by 512 (one PSUM bank per matmul),
    e/o halves of a 2-bank pair tile.
  - relu(conv) from PSUM by ScalarE (activation Relu) and VectorE (tensor_scalar
    max) in parallel, written to SBUF as float16, one instr per pair (FD=1024).
  - window-sum via matmul with 0/1 selection matrices G (128,32) f16, K=128:
    tile t feeds output column-group t//7 (4 col-groups run concurrently);
    7 tiles accumulate per group into one (128,512) PSUM bank per chunk.
  - a flat software pipeline over (chunk, burst) units: conv bursts of 3 pairs,
    window-sum bursts lag by 3 units, PSUM->SBUF casts lag 1 more unit. This
    keeps LDWEIGHTS hidden inside homogeneous bursts (conv LDWs alternate PE
    row-groups, sum LDWs touch disjoint col-groups) and removes chunk-boundary
    stalls on every engine.
  - output staged as f16, DMA'd per chunk; host casts to f32.
"""

import os
import sys

import numpy as np
from numpy.lib.stride_tricks import sliding_window_view

for _p in ("/opt/trn_rl_repo", "/root/.axon_site/_ro/trn_rl_repo"):
    if os.path.isdir(_p) and _p not in sys.path:
        sys.path.insert(0, _p)

import concourse.bacc as bacc
import concourse.mybir as mybir
from concourse.tile import TileContext
from concourse.bass_utils import run_bass_kernel_spmd

# problem sizes (hardcoded per spec)
N_, M_, L_, B_ = 32, 32, 30, 21
A_, K_ = 20, 3
F_ = 8000
NM = N_ * M_              # 1024
CORES = 8
NMC = NM // CORES         # 128 rows per core
LOUT = L_ - K_ + 1        # 28
FLAT = NMC * LOUT         # 3584 (nm,l) positions per core
NT = FLAT // 128          # 28 tiles of 128 positions
NP = NT // 2              # 14 e/o pairs
TPG = NT // 4             # 7 tiles accumulate per output col group
KC = A_ * K_ + 1          # 61 = 60 + bias row
FCH = 512                 # f chunk (one PSUM bank per conv matmul)
NCH = (F_ + FCH - 1) // FCH   # 16 chunks (last is 320 wide)

_f32 = mybir.dt.float32
_f16 = mybir.dt.float16

# pair bursts: convs for burst u and window-sums for burst u-2 are emitted as
# separate groups so LDWEIGHTS can pull ahead within each group. Each burst
# spans 3-4 distinct sum col-groups to keep the 4-way sum concurrency.
PAIR_BURSTS = [(0, 4, 7), (11, 1, 5), (8, 12, 2), (6, 9, 13), (3, 10)]
NB = len(PAIR_BURSTS)
PAIR_EMIT = [q for burst in PAIR_BURSTS for q in burst]   # emission order

_cached_nc = None


def _build_program():
    nc = bacc.Bacc("TRN2", target_bir_lowering=False, debug=False,
                   num_devices=CORES)
    xin_d = nc.declare_dram_parameter("xin", [128, NP * 128], _f16,
                                      isOutput=False)
    wt_d = nc.declare_dram_parameter("wt", [128, F_], _f16, isOutput=False)
    g_d = nc.declare_dram_parameter("g", [128, NT * 32], _f16, isOutput=False)
    out_d = nc.declare_dram_parameter("out", [NMC, F_], _f16, isOutput=True)

    relu_fn = mybir.ActivationFunctionType.Relu
    max_op = mybir.AluOpType.max

    with TileContext(nc) as tc:
        with tc.tile_pool(name="const", bufs=1) as cpool, \
             tc.tile_pool(name="rbuf", bufs=15) as rpool, \
             tc.tile_pool(name="stage", bufs=1) as spool, \
             tc.tile_pool(name="pconv", bufs=3, space="PSUM") as pconv, \
             tc.tile_pool(name="psum", bufs=2, space="PSUM") as psump:
            xin_sb = cpool.tile([128, NP * 128], _f16)
            wt_sb = cpool.tile([128, F_], _f16)
            g_sb = cpool.tile([128, NT * 32], _f16)
            stage = spool.tile([NMC, F_], _f16)
            scratch = cpool.tile([64, 512], _f16)

            # PE warmup: ~3.4us of dummy matmuls during the input-DMA wait so
            # the HAM clock gate is already at 8/8 when the real convs start
            nc.gpsimd.memset(scratch[:], 0.0)
            warm = pconv.tile([128, 1024], _f32, tag="pc")
            for _ in range(8):
                nc.tensor.matmul(out=warm[:, 0:512],
                                 lhsT=scratch[0:KC, 0:128],
                                 rhs=scratch[0:KC, 0:512],
                                 start=True, stop=True)
            # preload order: first conv burst is runnable early (xin is laid
            # out in burst-emission order, so its first 384 cols come first)
            nc.sync.dma_start(out=wt_sb[:, 0:FCH], in_=wt_d[:, 0:FCH])
            nc.sync.dma_start(out=xin_sb[:, 0:384], in_=xin_d[:, 0:384])
            nc.sync.dma_start(out=xin_sb[:, 384:NP * 128],
                              in_=xin_d[:, 384:NP * 128])
            nc.sync.dma_start(out=g_sb[:], in_=g_d[:])
            for cw in range(1, NCH):
                s = slice(cw * FCH, min(F_, (cw + 1) * FCH))
                nc.sync.dma_start(out=wt_sb[:, s], in_=wt_d[:, s])

            units = [(c, b) for c in range(NCH) for b in range(NB)]
            pend = []        # per unit: (chunk, [(tile pair, r tile), ...])
            ps_of = {}       # chunk -> accumulating sum tile
            nvisit = {}      # chunk -> per-col-group visit counts
            casts = []       # chunks whose sums are done, cast not yet emitted

            def do_sums(cu, qrs):
                if cu not in ps_of:
                    ps_of[cu] = psump.tile([128, 512], _f32, tag="ps",
                                           name=f"ps{cu}")
                    nvisit[cu] = [0, 0, 0, 0]
                w = min(F_, (cu + 1) * FCH) - cu * FCH
                for q, r in qrs:
                    for t, rs in ((2 * q, slice(0, w)),
                                  (2 * q + 1, slice(512, 512 + w))):
                        grp = t // TPG
                        nc.tensor.matmul(
                            out=ps_of[cu][32 * grp:32 * grp + 32, 0:w],
                            lhsT=g_sb[:, 32 * t:32 * t + 32],
                            rhs=r[:, rs],
                            start=(nvisit[cu][grp] == 0),
                            stop=(nvisit[cu][grp] == TPG - 1),
                            skip_group_check=True,
                            tile_position=(0, 32 * grp))
                        nvisit[cu][grp] += 1

            def do_cast(cu):
                fs = slice(cu * FCH, min(F_, (cu + 1) * FCH))
                w = fs.stop - fs.start
                nc.scalar.copy(out=stage[:, fs], in_=ps_of.pop(cu)[:, 0:w])
                del nvisit[cu]
                nc.sync.dma_start(out=out_d[:, fs], in_=stage[:, fs])

            for u, (c, b) in enumerate(units):
                fs = slice(c * FCH, min(F_, (c + 1) * FCH))
                w = fs.stop - fs.start
                qrs = []
                for j, q in enumerate(PAIR_BURSTS[b]):
                    p_loc = 3 * b + j        # chunk-local emission position
                    xo = p_loc * 128
                    pc = pconv.tile([128, 1024], _f32, tag="pc")
                    nc.tensor.matmul(
                        out=pc[:, 0:w],
                        lhsT=xin_sb[0:KC, xo:xo + 128],
                        rhs=wt_sb[0:KC, fs], start=True, stop=True)
                    nc.tensor.matmul(
                        out=pc[:, 512:512 + w],
                        lhsT=xin_sb[64:64 + KC, xo:xo + 128],
                        rhs=wt_sb[64:64 + KC, fs],
                        start=True, stop=True)
                    r = rpool.tile([128, 1024], _f16, tag="r")
                    # relu split: 7 on vector, 7 + per-chunk cast on scalar;
                    # chunk 0 runs 6/8 (scalar has slack during pipeline ramp)
                    if (p_loc % 2 == 0 if c else p_loc % 2 == 1 and p_loc < 13):
                        nc.vector.tensor_scalar(out=r[:, 0:512 + w],
                                                in0=pc[:, 0:512 + w],
                                                scalar1=0.0, scalar2=None,
                                                op0=max_op)
                    else:
                        nc.scalar.activation(out=r[:, 0:512 + w],
                                             in_=pc[:, 0:512 + w],
                                             func=relu_fn)
                    qrs.append((q, r))
                pend.append((c, qrs))
                if casts:
                    do_cast(casts.pop(0))
                if u >= 3:
                    cu, qrs_u = pend.pop(0)
                    do_sums(cu, qrs_u)
                    if (u - 3) % NB == NB - 1:
                        casts.append(cu)
            for cu, qrs_u in pend:
                do_sums(cu, qrs_u)
                if casts:
                    do_cast(casts.pop(0))
            casts.append(NCH - 1)
            while casts:
                do_cast(casts.pop(0))

    nc.compile()
    return nc


def _get_program():
    global _cached_nc
    if _cached_nc is None:
        _cached_nc = _build_program()
    return _cached_nc


def _host_prep(seqs, weight, bias):
    s = np.asarray(seqs, np.float32).reshape(NM, L_, B_)[:, :, :A_]
    sw = sliding_window_view(s, K_, axis=1)          # (NM, 28, 20, 3)
    X = sw.transpose(3, 2, 0, 1).reshape(A_ * K_, NM, LOUT)
    X = np.concatenate([X, np.ones((1, NM, LOUT), np.float32)], axis=0)

    Wt = np.asarray(weight, np.float32).transpose(2, 1, 0).reshape(A_ * K_, F_)
    Wb = np.concatenate([Wt, np.asarray(bias, np.float32)[None, :]], axis=0)
    wt = np.zeros((128, F_), np.float32)
    wt[0:KC] = Wb
    wt[64:64 + KC] = Wb
    wt_f16 = wt.astype(np.float16)

    # G_t[j, m] = 1 iff position 128t+j belongs to output row m of col group t//7
    G = np.zeros((128, NT * 32), np.float16)
    for t in range(NT):
        nm_of_j = (128 * t + np.arange(128)) // LOUT
        G[np.arange(128), 32 * t + nm_of_j % 32] = 1.0

    in_maps = []
    for c in range(CORES):
        Xc = X[:, c * NMC:(c + 1) * NMC, :].reshape(KC, NT, 128)
        xin = np.zeros((128, NP, 128), np.float32)
        xin[0:KC] = Xc[:, [2 * q for q in PAIR_EMIT]]
        xin[64:64 + KC] = Xc[:, [2 * q + 1 for q in PAIR_EMIT]]
        in_maps.append({
            "xin": np.ascontiguousarray(
                xin.reshape(128, NP * 128)).astype(np.float16),
            "wt": wt_f16,
            "g": G,
        })
    return in_maps


def run_bass(seqs, weight, bias, trace=False):
    """Returns (out (32,32,8000) float32, exec_time_ns or None)."""
    nc = _get_program()
    in_maps = _host_prep(seqs, weight, bias)
    res = run_bass_kernel_spmd(nc, in_maps, list(range(CORES)), trace=trace)
    out = np.concatenate([res.results[c]["out"] for c in range(CORES)], axis=0)
    return out.reshape(N_, M_, F_).astype(np.float32), res.exec_time_ns


def kernel(seqs, weight, bias):
    out, _ = run_bass(seqs, weight, bias, trace=False)
    return out


# revision 42
# speedup vs baseline: 1.0057x; 1.0057x over previous
"""Trainium2 Bass kernel for nn_KmerEmbed: conv1d(one-hot kmer filters) + relu + window-sum.

Computes, for seqs (32,32,30,21), weight (8000,20,3), bias (8000,):
  out[n,m,f] = sum_l relu( sum_{a,j} seqs[n,m,l+j,a(<20)]*weight[f,a,j] + bias[f] )
with l over the 28 valid conv positions; returns (32,32,8000) float32.

Strategy (8 NeuronCores, data-parallel over the 1024 flattened rows, 128 rows/core):
  - im2col on host: the 128 rows x 28 positions = 3584 (nm,l) pairs are packed
    densely into 28 tiles of 128 PSUM partitions (full PE width), K=61 rows
    (60 one-hot taps + bias row).
  - conv = matmul vs the replicated filter matrix Wb (61,8000) f16; tiles are
    packed in pairs into PE row-groups (partitions 0-60 / 64-124) so two
    matmuls stream concurrently; f chunked by 512 (one PSUM bank per matmul),
    e/o halves of a 2-bank pair tile.
  - relu(conv) from PSUM by ScalarE (activation Relu) and VectorE (tensor_scalar
    max) in parallel, written to SBUF as float16, one instr per pair (FD=1024).
  - window-sum via matmul with 0/1 selection matrices G (128,32) f16, K=128:
    tile t feeds output column-group t//7 (4 col-groups run concurrently);
    7 tiles accumulate per group into one (128,512) PSUM bank per chunk.
  - a flat software pipeline over (chunk, burst) units: conv bursts of 3 pairs,
    window-sum bursts lag by 3 units, PSUM->SBUF casts lag 1 more unit. This
    keeps LDWEIGHTS hidden inside homogeneous bursts (conv LDWs alternate PE
    row-groups, sum LDWs touch disjoint col-groups) and removes chunk-boundary
    stalls on every engine.
  - output staged as f16, DMA'd per chunk; host casts to f32.
"""

import os
import sys

import numpy as np
from numpy.lib.stride_tricks import sliding_window_view

for _p in ("/opt/trn_rl_repo", "/root/.axon_site/_ro/trn_rl_repo"):
    if os.path.isdir(_p) and _p not in sys.path:
        sys.path.insert(0, _p)

import concourse.bacc as bacc
import concourse.mybir as mybir
from concourse.tile import TileContext
from concourse.bass_utils import run_bass_kernel_spmd

# problem sizes (hardcoded per spec)
N_, M_, L_, B_ = 32, 32, 30, 21
A_, K_ = 20, 3
F_ = 8000
NM = N_ * M_              # 1024
CORES = 8
NMC = NM // CORES         # 128 rows per core
LOUT = L_ - K_ + 1        # 28
FLAT = NMC * LOUT         # 3584 (nm,l) positions per core
NT = FLAT // 128          # 28 tiles of 128 positions
NP = NT // 2              # 14 e/o pairs
TPG = NT // 4             # 7 tiles accumulate per output col group
KC = A_ * K_ + 1          # 61 = 60 + bias row
FCH = 512                 # f chunk (one PSUM bank per conv matmul)
NCH = (F_ + FCH - 1) // FCH   # 16 chunks (last is 320 wide)

_f32 = mybir.dt.float32
_f16 = mybir.dt.float16

# pair bursts: convs for burst u and window-sums for burst u-2 are emitted as
# separate groups so LDWEIGHTS can pull ahead within each group. Each burst
# spans 3-4 distinct sum col-groups to keep the 4-way sum concurrency.
PAIR_BURSTS = [(0, 4, 7), (11, 1, 5), (8, 12, 2), (6, 9, 13), (3, 10)]
NB = len(PAIR_BURSTS)
PAIR_EMIT = [q for burst in PAIR_BURSTS for q in burst]   # emission order

_cached_nc = None


def _build_program():
    nc = bacc.Bacc("TRN2", target_bir_lowering=False, debug=False,
                   num_devices=CORES)
    xin_d = nc.declare_dram_parameter("xin", [128, NP * 128], _f16,
                                      isOutput=False)
    wt_d = nc.declare_dram_parameter("wt", [128, F_], _f16, isOutput=False)
    g_d = nc.declare_dram_parameter("g", [128, NT * 32], _f16, isOutput=False)
    out_d = nc.declare_dram_parameter("out", [NMC, F_], _f16, isOutput=True)

    relu_fn = mybir.ActivationFunctionType.Relu
    max_op = mybir.AluOpType.max

    with TileContext(nc) as tc:
        with tc.tile_pool(name="const", bufs=1) as cpool, \
             tc.tile_pool(name="rbuf", bufs=15) as rpool, \
             tc.tile_pool(name="stage", bufs=1) as spool, \
             tc.tile_pool(name="pconv", bufs=3, space="PSUM") as pconv, \
             tc.tile_pool(name="psum", bufs=2, space="PSUM") as psump:
            xin_sb = cpool.tile([128, NP * 128], _f16)
            wt_sb = cpool.tile([128, F_], _f16)
            g_sb = cpool.tile([128, NT * 32], _f16)
            stage = spool.tile([NMC, F_], _f16)
            # preload order: first conv burst is runnable early (xin is laid
            # out in burst-emission order, so its first 384 cols come first)
            nc.sync.dma_start(out=wt_sb[:, 0:FCH], in_=wt_d[:, 0:FCH])
            nc.sync.dma_start(out=xin_sb[:, 0:384], in_=xin_d[:, 0:384])
            nc.sync.dma_start(out=xin_sb[:, 384:NP * 128],
                              in_=xin_d[:, 384:NP * 128])
            nc.sync.dma_start(out=g_sb[:], in_=g_d[:])
            for cw in range(1, NCH):
                s = slice(cw * FCH, min(F_, (cw + 1) * FCH))
                nc.sync.dma_start(out=wt_sb[:, s], in_=wt_d[:, s])

            units = [(c, b) for c in range(NCH) for b in range(NB)]
            pend = []        # per unit: (chunk, [(tile pair, r tile), ...])
            ps_of = {}       # chunk -> accumulating sum tile
            nvisit = {}      # chunk -> per-col-group visit counts
            casts = []       # chunks whose sums are done, cast not yet emitted

            def do_sums(cu, qrs):
                if cu not in ps_of:
                    ps_of[cu] = psump.tile([128, 512], _f32, tag="ps",
                                           name=f"ps{cu}")
                    nvisit[cu] = [0, 0, 0, 0]
                w = min(F_, (cu + 1) * FCH) - cu * FCH
                for q, r in qrs:
                    for t, rs in ((2 * q, slice(0, w)),
                                  (2 * q + 1, slice(512, 512 + w))):
                        grp = t // TPG
                        nc.tensor.matmul(
                            out=ps_of[cu][32 * grp:32 * grp + 32, 0:w],
                            lhsT=g_sb[:, 32 * t:32 * t + 32],
                            rhs=r[:, rs],
                            start=(nvisit[cu][grp] == 0),
                            stop=(nvisit[cu][grp] == TPG - 1),
                            skip_group_check=True,
                            tile_position=(0, 32 * grp))
                        nvisit[cu][grp] += 1

            def do_cast(cu):
                fs = slice(cu * FCH, min(F_, (cu + 1) * FCH))
                w = fs.stop - fs.start
                nc.scalar.copy(out=stage[:, fs], in_=ps_of.pop(cu)[:, 0:w])
                del nvisit[cu]
                nc.sync.dma_start(out=out_d[:, fs], in_=stage[:, fs])

            for u, (c, b) in enumerate(units):
                fs = slice(c * FCH, min(F_, (c + 1) * FCH))
                w = fs.stop - fs.start
                qrs = []
                for j, q in enumerate(PAIR_BURSTS[b]):
                    p_loc = 3 * b + j        # chunk-local emission position
                    xo = p_loc * 128
                    pc = pconv.tile([128, 1024], _f32, tag="pc")
                    nc.tensor.matmul(
                        out=pc[:, 0:w],
                        lhsT=xin_sb[0:KC, xo:xo + 128],
                        rhs=wt_sb[0:KC, fs], start=True, stop=True)
                    nc.tensor.matmul(
                        out=pc[:, 512:512 + w],
                        lhsT=xin_sb[64:64 + KC, xo:xo + 128],
                        rhs=wt_sb[64:64 + KC, fs],
                        start=True, stop=True)
                    r = rpool.tile([128, 1024], _f16, tag="r")
                    # relu split: 7 on vector, 7 + per-chunk cast on scalar;
                    # chunk 0 runs 6/8 (scalar has slack during pipeline ramp)
                    if (p_loc % 2 == 0 if c else p_loc % 2 == 1 and p_loc < 13):
                        nc.vector.tensor_scalar(out=r[:, 0:512 + w],
                                                in0=pc[:, 0:512 + w],
                                                scalar1=0.0, scalar2=None,
                                                op0=max_op)
                    else:
                        nc.scalar.activation(out=r[:, 0:512 + w],
                                             in_=pc[:, 0:512 + w],
                                             func=relu_fn)
                    qrs.append((q, r))
                pend.append((c, qrs))
                if casts:
                    do_cast(casts.pop(0))
                if u >= 3:
                    cu, qrs_u = pend.pop(0)
                    do_sums(cu, qrs_u)
                    if (u - 3) % NB == NB - 1:
                        casts.append(cu)
            for cu, qrs_u in pend:
                do_sums(cu, qrs_u)
                if casts:
                    do_cast(casts.pop(0))
            casts.append(NCH - 1)
            while casts:
                do_cast(casts.pop(0))

    nc.compile()
    return nc


def _get_program():
    global _cached_nc
    if _cached_nc is None:
        _cached_nc = _build_program()
    return _cached_nc


def _host_prep(seqs, weight, bias):
    s = np.asarray(seqs, np.float32).reshape(NM, L_, B_)[:, :, :A_]
    sw = sliding_window_view(s, K_, axis=1)          # (NM, 28, 20, 3)
    X = sw.transpose(3, 2, 0, 1).reshape(A_ * K_, NM, LOUT)
    X = np.concatenate([X, np.ones((1, NM, LOUT), np.float32)], axis=0)

    Wt = np.asarray(weight, np.float32).transpose(2, 1, 0).reshape(A_ * K_, F_)
    Wb = np.concatenate([Wt, np.asarray(bias, np.float32)[None, :]], axis=0)
    wt = np.zeros((128, F_), np.float32)
    wt[0:KC] = Wb
    wt[64:64 + KC] = Wb
    wt_f16 = wt.astype(np.float16)

    # G_t[j, m] = 1 iff position 128t+j belongs to output row m of col group t//7
    G = np.zeros((128, NT * 32), np.float16)
    for t in range(NT):
        nm_of_j = (128 * t + np.arange(128)) // LOUT
        G[np.arange(128), 32 * t + nm_of_j % 32] = 1.0

    in_maps = []
    for c in range(CORES):
        Xc = X[:, c * NMC:(c + 1) * NMC, :].reshape(KC, NT, 128)
        xin = np.zeros((128, NP, 128), np.float32)
        xin[0:KC] = Xc[:, [2 * q for q in PAIR_EMIT]]
        xin[64:64 + KC] = Xc[:, [2 * q + 1 for q in PAIR_EMIT]]
        in_maps.append({
            "xin": np.ascontiguousarray(
                xin.reshape(128, NP * 128)).astype(np.float16),
            "wt": wt_f16,
            "g": G,
        })
    return in_maps


def run_bass(seqs, weight, bias, trace=False):
    """Returns (out (32,32,8000) float32, exec_time_ns or None)."""
    nc = _get_program()
    in_maps = _host_prep(seqs, weight, bias)
    res = run_bass_kernel_spmd(nc, in_maps, list(range(CORES)), trace=trace)
    out = np.concatenate([res.results[c]["out"] for c in range(CORES)], axis=0)
    return out.reshape(N_, M_, F_).astype(np.float32), res.exec_time_ns


def kernel(seqs, weight, bias):
    out, _ = run_bass(seqs, weight, bias, trace=False)
    return out


# revision 46
# speedup vs baseline: 1.0075x; 1.0018x over previous
"""Trainium2 Bass kernel for nn_KmerEmbed: conv1d(one-hot kmer filters) + relu + window-sum.

Computes, for seqs (32,32,30,21), weight (8000,20,3), bias (8000,):
  out[n,m,f] = sum_l relu( sum_{a,j} seqs[n,m,l+j,a(<20)]*weight[f,a,j] + bias[f] )
with l over the 28 valid conv positions; returns (32,32,8000) float32.

Strategy (8 NeuronCores, data-parallel over the 1024 flattened rows, 128 rows/core):
  - im2col on host: the 128 rows x 28 positions = 3584 (nm,l) pairs are packed
    densely into 28 tiles of 128 PSUM partitions (full PE width), K=61 rows
    (60 one-hot taps + bias row).
  - conv = matmul vs the replicated filter matrix Wb (61,8000) f16; tiles are
    packed in pairs into PE row-groups (partitions 0-60 / 64-124) so two
    matmuls stream concurrently; f chunked by 512 (one PSUM bank per matmul),
    e/o halves of a 2-bank pair tile.
  - relu(conv) from PSUM by ScalarE (activation Relu) and VectorE (tensor_scalar
    max) in parallel, written to SBUF as float16, one instr per pair (FD=1024).
  - window-sum via matmul with 0/1 selection matrices G (128,32) f16, K=128:
    tile t feeds output column-group t//7 (4 col-groups run concurrently);
    7 tiles accumulate per group into one (128,512) PSUM bank per chunk.
  - a flat software pipeline over (chunk, burst) units: conv bursts of 3 pairs,
    window-sum bursts lag by 3 units, PSUM->SBUF casts lag 1 more unit. This
    keeps LDWEIGHTS hidden inside homogeneous bursts (conv LDWs alternate PE
    row-groups, sum LDWs touch disjoint col-groups) and removes chunk-boundary
    stalls on every engine.
  - output staged as f16, DMA'd per chunk; host casts to f32.
"""

import os
import sys

import numpy as np
from numpy.lib.stride_tricks import sliding_window_view

for _p in ("/opt/trn_rl_repo", "/root/.axon_site/_ro/trn_rl_repo"):
    if os.path.isdir(_p) and _p not in sys.path:
        sys.path.insert(0, _p)

import concourse.bacc as bacc
import concourse.mybir as mybir
from concourse.tile import TileContext
from concourse.bass_utils import run_bass_kernel_spmd

# problem sizes (hardcoded per spec)
N_, M_, L_, B_ = 32, 32, 30, 21
A_, K_ = 20, 3
F_ = 8000
NM = N_ * M_              # 1024
CORES = 8
NMC = NM // CORES         # 128 rows per core
LOUT = L_ - K_ + 1        # 28
FLAT = NMC * LOUT         # 3584 (nm,l) positions per core
NT = FLAT // 128          # 28 tiles of 128 positions
NP = NT // 2              # 14 e/o pairs
TPG = NT // 4             # 7 tiles accumulate per output col group
KC = A_ * K_ + 1          # 61 = 60 + bias row
FCH = 512                 # f chunk (one PSUM bank per conv matmul)
NCH = (F_ + FCH - 1) // FCH   # 16 chunks (last is 320 wide)

_f32 = mybir.dt.float32
_f16 = mybir.dt.float16

# pair bursts: convs for burst u and window-sums for burst u-2 are emitted as
# separate groups so LDWEIGHTS can pull ahead within each group. Each burst
# spans 3-4 distinct sum col-groups to keep the 4-way sum concurrency.
PAIR_BURSTS = [(0, 4, 7), (11, 1, 5), (8, 12, 2), (6, 9, 13), (3, 10)]
NB = len(PAIR_BURSTS)
PAIR_EMIT = [q for burst in PAIR_BURSTS for q in burst]   # emission order

_cached_nc = None


def _build_program():
    nc = bacc.Bacc("TRN2", target_bir_lowering=False, debug=False,
                   num_devices=CORES)
    xin_d = nc.declare_dram_parameter("xin", [128, NP * 128], _f16,
                                      isOutput=False)
    wt_d = nc.declare_dram_parameter("wt", [128, F_], _f16, isOutput=False)
    g_d = nc.declare_dram_parameter("g", [128, NT * 32], _f16, isOutput=False)
    # boot tensor: [xin emission positions 0-5 | wt chunk 0] packed so ONE
    # 2.5KB-per-partition DMA (vs two sub-1KB ones at ~49GB/s) lands
    # everything the first two conv bursts need ~2us earlier
    boot_d = nc.declare_dram_parameter("boot", [128, 1280], _f16,
                                       isOutput=False)
    out_d = nc.declare_dram_parameter("out", [NMC, F_], _f16, isOutput=True)

    relu_fn = mybir.ActivationFunctionType.Relu
    max_op = mybir.AluOpType.max

    with TileContext(nc) as tc:
        with tc.tile_pool(name="const", bufs=1) as cpool, \
             tc.tile_pool(name="rbuf", bufs=15) as rpool, \
             tc.tile_pool(name="stage", bufs=1) as spool, \
             tc.tile_pool(name="pconv", bufs=3, space="PSUM") as pconv, \
             tc.tile_pool(name="psum", bufs=2, space="PSUM") as psump:
            xin_sb = cpool.tile([128, NP * 128], _f16)
            wt_sb = cpool.tile([128, F_], _f16)
            g_sb = cpool.tile([128, NT * 32], _f16)
            boot_sb = cpool.tile([128, 1280], _f16)
            stage = spool.tile([NMC, F_], _f16)
            nc.sync.dma_start(out=boot_sb[:], in_=boot_d[:])
            nc.sync.dma_start(out=xin_sb[:, 768:NP * 128],
                              in_=xin_d[:, 768:NP * 128])
            nc.sync.dma_start(out=g_sb[:], in_=g_d[:])
            for cw in range(1, NCH):
                s = slice(cw * FCH, min(F_, (cw + 1) * FCH))
                nc.sync.dma_start(out=wt_sb[:, s], in_=wt_d[:, s])

            units = [(c, b) for c in range(NCH) for b in range(NB)]
            pend = []        # per unit: (chunk, [(tile pair, r tile), ...])
            ps_of = {}       # chunk -> accumulating sum tile
            nvisit = {}      # chunk -> per-col-group visit counts
            casts = []       # chunks whose sums are done, cast not yet emitted

            def do_sums(cu, qrs):
                if cu not in ps_of:
                    ps_of[cu] = psump.tile([128, 512], _f32, tag="ps",
                                           name=f"ps{cu}")
                    nvisit[cu] = [0, 0, 0, 0]
                w = min(F_, (cu + 1) * FCH) - cu * FCH
                for q, r in qrs:
                    for t, rs in ((2 * q, slice(0, w)),
                                  (2 * q + 1, slice(512, 512 + w))):
                        grp = t // TPG
                        nc.tensor.matmul(
                            out=ps_of[cu][32 * grp:32 * grp + 32, 0:w],
                            lhsT=g_sb[:, 32 * t:32 * t + 32],
                            rhs=r[:, rs],
                            start=(nvisit[cu][grp] == 0),
                            stop=(nvisit[cu][grp] == TPG - 1),
                            skip_group_check=True,
                            tile_position=(0, 32 * grp))
                        nvisit[cu][grp] += 1

            def do_cast(cu):
                fs = slice(cu * FCH, min(F_, (cu + 1) * FCH))
                w = fs.stop - fs.start
                nc.scalar.copy(out=stage[:, fs], in_=ps_of.pop(cu)[:, 0:w])
                del nvisit[cu]
                nc.sync.dma_start(out=out_d[:, fs], in_=stage[:, fs])

            for u, (c, b) in enumerate(units):
                fs = slice(c * FCH, min(F_, (c + 1) * FCH))
                w = fs.stop - fs.start
                qrs = []
                for j, q in enumerate(PAIR_BURSTS[b]):
                    p_loc = 3 * b + j        # chunk-local emission position
                    xo = p_loc * 128
                    xb = boot_sb if p_loc < 6 else xin_sb
                    if c == 0:
                        wb, ws = boot_sb, slice(768, 768 + w)
                    else:
                        wb, ws = wt_sb, fs
                    pc = pconv.tile([128, 1024], _f32, tag="pc")
                    nc.tensor.matmul(
                        out=pc[:, 0:w],
                        lhsT=xb[0:KC, xo:xo + 128],
                        rhs=wb[0:KC, ws], start=True, stop=True)
                    nc.tensor.matmul(
                        out=pc[:, 512:512 + w],
                        lhsT=xb[64:64 + KC, xo:xo + 128],
                        rhs=wb[64:64 + KC, ws],
                        start=True, stop=True)
                    r = rpool.tile([128, 1024], _f16, tag="r")
                    # relu split: 7 on vector, 7 + per-chunk cast on scalar;
                    # chunk 0 runs 6/8 (scalar has slack during pipeline ramp)
                    if (p_loc % 2 == 0 if c else p_loc % 2 == 1 and p_loc < 13):
                        nc.vector.tensor_scalar(out=r[:, 0:512 + w],
                                                in0=pc[:, 0:512 + w],
                                                scalar1=0.0, scalar2=None,
                                                op0=max_op)
                    else:
                        nc.scalar.activation(out=r[:, 0:512 + w],
                                             in_=pc[:, 0:512 + w],
                                             func=relu_fn)
                    qrs.append((q, r))
                pend.append((c, qrs))
                if casts:
                    do_cast(casts.pop(0))
                if u >= 3:
                    cu, qrs_u = pend.pop(0)
                    do_sums(cu, qrs_u)
                    if (u - 3) % NB == NB - 1:
                        casts.append(cu)
            for cu, qrs_u in pend:
                do_sums(cu, qrs_u)
                if casts:
                    do_cast(casts.pop(0))
            casts.append(NCH - 1)
            while casts:
                do_cast(casts.pop(0))

    nc.compile()
    return nc


def _get_program():
    global _cached_nc
    if _cached_nc is None:
        _cached_nc = _build_program()
    return _cached_nc


def _host_prep(seqs, weight, bias):
    s = np.asarray(seqs, np.float32).reshape(NM, L_, B_)[:, :, :A_]
    sw = sliding_window_view(s, K_, axis=1)          # (NM, 28, 20, 3)
    X = sw.transpose(3, 2, 0, 1).reshape(A_ * K_, NM, LOUT)
    X = np.concatenate([X, np.ones((1, NM, LOUT), np.float32)], axis=0)

    Wt = np.asarray(weight, np.float32).transpose(2, 1, 0).reshape(A_ * K_, F_)
    Wb = np.concatenate([Wt, np.asarray(bias, np.float32)[None, :]], axis=0)
    wt = np.zeros((128, F_), np.float32)
    wt[0:KC] = Wb
    wt[64:64 + KC] = Wb
    wt_f16 = wt.astype(np.float16)

    # G_t[j, m] = 1 iff position 128t+j belongs to output row m of col group t//7
    G = np.zeros((128, NT * 32), np.float16)
    for t in range(NT):
        nm_of_j = (128 * t + np.arange(128)) // LOUT
        G[np.arange(128), 32 * t + nm_of_j % 32] = 1.0

    in_maps = []
    for c in range(CORES):
        Xc = X[:, c * NMC:(c + 1) * NMC, :].reshape(KC, NT, 128)
        xin = np.zeros((128, NP, 128), np.float32)
        xin[0:KC] = Xc[:, [2 * q for q in PAIR_EMIT]]
        xin[64:64 + KC] = Xc[:, [2 * q + 1 for q in PAIR_EMIT]]
        xin = np.ascontiguousarray(
            xin.reshape(128, NP * 128)).astype(np.float16)
        boot = np.ascontiguousarray(
            np.concatenate([xin[:, 0:768], wt_f16[:, 0:FCH]], axis=1))
        in_maps.append({"xin": xin, "wt": wt_f16, "g": G, "boot": boot})
    return in_maps


def run_bass(seqs, weight, bias, trace=False):
    """Returns (out (32,32,8000) float32, exec_time_ns or None)."""
    nc = _get_program()
    in_maps = _host_prep(seqs, weight, bias)
    res = run_bass_kernel_spmd(nc, in_maps, list(range(CORES)), trace=trace)
    out = np.concatenate([res.results[c]["out"] for c in range(CORES)], axis=0)
    return out.reshape(N_, M_, F_).astype(np.float32), res.exec_time_ns


def kernel(seqs, weight, bias):
    out, _ = run_bass(seqs, weight, bias, trace=False)
    return out


# revision 47
# speedup vs baseline: 1.0113x; 1.0037x over previous
"""Trainium2 Bass kernel for nn_KmerEmbed: conv1d(one-hot kmer filters) + relu + window-sum.

Computes, for seqs (32,32,30,21), weight (8000,20,3), bias (8000,):
  out[n,m,f] = sum_l relu( sum_{a,j} seqs[n,m,l+j,a(<20)]*weight[f,a,j] + bias[f] )
with l over the 28 valid conv positions; returns (32,32,8000) float32.

Strategy (8 NeuronCores, data-parallel over the 1024 flattened rows, 128 rows/core):
  - im2col on host: the 128 rows x 28 positions = 3584 (nm,l) pairs are packed
    densely into 28 tiles of 128 PSUM partitions (full PE width), K=61 rows
    (60 one-hot taps + bias row).
  - conv = matmul vs the replicated filter matrix Wb (61,8000) f16; tiles are
    packed in pairs into PE row-groups (partitions 0-60 / 64-124) so two
    matmuls stream concurrently; f chunked by 512 (one PSUM bank per matmul),
    e/o halves of a 2-bank pair tile.
  - relu(conv) from PSUM by ScalarE (activation Relu) and VectorE (tensor_scalar
    max) in parallel, written to SBUF as float16, one instr per pair (FD=1024).
  - window-sum via matmul with 0/1 selection matrices G (128,32) f16, K=128:
    tile t feeds output column-group t//7 (4 col-groups run concurrently);
    7 tiles accumulate per group into one (128,512) PSUM bank per chunk.
  - a flat software pipeline over (chunk, burst) units: conv bursts of 3 pairs,
    window-sum bursts lag by 3 units, PSUM->SBUF casts lag 1 more unit. This
    keeps LDWEIGHTS hidden inside homogeneous bursts (conv LDWs alternate PE
    row-groups, sum LDWs touch disjoint col-groups) and removes chunk-boundary
    stalls on every engine.
  - output staged as f16, DMA'd per chunk; host casts to f32.
"""

import os
import sys

import numpy as np
from numpy.lib.stride_tricks import sliding_window_view

for _p in ("/opt/trn_rl_repo", "/root/.axon_site/_ro/trn_rl_repo"):
    if os.path.isdir(_p) and _p not in sys.path:
        sys.path.insert(0, _p)

import concourse.bacc as bacc
import concourse.mybir as mybir
from concourse.tile import TileContext
from concourse.bass_utils import run_bass_kernel_spmd

# problem sizes (hardcoded per spec)
N_, M_, L_, B_ = 32, 32, 30, 21
A_, K_ = 20, 3
F_ = 8000
NM = N_ * M_              # 1024
CORES = 8
NMC = NM // CORES         # 128 rows per core
LOUT = L_ - K_ + 1        # 28
FLAT = NMC * LOUT         # 3584 (nm,l) positions per core
NT = FLAT // 128          # 28 tiles of 128 positions
NP = NT // 2              # 14 e/o pairs
TPG = NT // 4             # 7 tiles accumulate per output col group
KC = A_ * K_ + 1          # 61 = 60 + bias row
FCH = 512                 # f chunk (one PSUM bank per conv matmul)
NCH = (F_ + FCH - 1) // FCH   # 16 chunks (last is 320 wide)

_f32 = mybir.dt.float32
_f16 = mybir.dt.float16

# pair bursts: convs for burst u and window-sums for burst u-2 are emitted as
# separate groups so LDWEIGHTS can pull ahead within each group. Each burst
# spans 3-4 distinct sum col-groups to keep the 4-way sum concurrency.
PAIR_BURSTS = [(0, 4, 7), (11, 1, 5), (8, 12, 2), (6, 9, 13), (3, 10)]
NB = len(PAIR_BURSTS)
PAIR_EMIT = [q for burst in PAIR_BURSTS for q in burst]   # emission order

_cached_nc = None


def _build_program():
    nc = bacc.Bacc("TRN2", target_bir_lowering=False, debug=False,
                   num_devices=CORES)
    xin_d = nc.declare_dram_parameter("xin", [128, NP * 128], _f16,
                                      isOutput=False)
    wt_d = nc.declare_dram_parameter("wt", [128, F_], _f16, isOutput=False)
    g_d = nc.declare_dram_parameter("g", [128, NT * 32], _f16, isOutput=False)
    # boot tensor: [xin emission positions 0-5 | wt chunk 0] packed so ONE
    # 2.5KB-per-partition DMA (vs two sub-1KB ones at ~49GB/s) lands
    # everything the first two conv bursts need ~2us earlier
    boot_d = nc.declare_dram_parameter("boot", [128, 1280], _f16,
                                       isOutput=False)
    out_d = nc.declare_dram_parameter("out", [NMC, F_], _f16, isOutput=True)

    relu_fn = mybir.ActivationFunctionType.Relu
    max_op = mybir.AluOpType.max

    with TileContext(nc) as tc:
        with tc.tile_pool(name="const", bufs=1) as cpool, \
             tc.tile_pool(name="rbuf", bufs=15) as rpool, \
             tc.tile_pool(name="stage", bufs=1) as spool, \
             tc.tile_pool(name="pconv", bufs=3, space="PSUM") as pconv, \
             tc.tile_pool(name="psum", bufs=2, space="PSUM") as psump:
            xin_sb = cpool.tile([128, NP * 128], _f16)
            wt_sb = cpool.tile([128, F_], _f16)
            g_sb = cpool.tile([128, NT * 32], _f16)
            boot_sb = cpool.tile([128, 1280], _f16)
            stage = spool.tile([NMC, F_], _f16)
            nc.sync.dma_start(out=boot_sb[:], in_=boot_d[:])
            nc.sync.dma_start(out=xin_sb[:, 768:NP * 128],
                              in_=xin_d[:, 768:NP * 128])
            nc.sync.dma_start(out=g_sb[:], in_=g_d[:])
            for cw in range(1, NCH):
                s = slice(cw * FCH, min(F_, (cw + 1) * FCH))
                nc.sync.dma_start(out=wt_sb[:, s], in_=wt_d[:, s])

            units = [(c, b) for c in range(NCH) for b in range(NB)]
            pend = []        # per unit: (chunk, [(tile pair, r tile), ...])
            ps_of = {}       # chunk -> accumulating sum tile
            nvisit = {}      # chunk -> per-col-group visit counts
            casts = []       # chunks whose sums are done, cast not yet emitted

            def do_sums(cu, qrs):
                if cu not in ps_of:
                    ps_of[cu] = psump.tile([128, 512], _f32, tag="ps",
                                           name=f"ps{cu}")
                    nvisit[cu] = [0, 0, 0, 0]
                w = min(F_, (cu + 1) * FCH) - cu * FCH
                for q, r in qrs:
                    for t, rs in ((2 * q, slice(0, w)),
                                  (2 * q + 1, slice(512, 512 + w))):
                        grp = t // TPG
                        nc.tensor.matmul(
                            out=ps_of[cu][32 * grp:32 * grp + 32, 0:w],
                            lhsT=g_sb[:, 32 * t:32 * t + 32],
                            rhs=r[:, rs],
                            start=(nvisit[cu][grp] == 0),
                            stop=(nvisit[cu][grp] == TPG - 1),
                            skip_group_check=True,
                            tile_position=(0, 32 * grp))
                        nvisit[cu][grp] += 1

            def do_cast(cu):
                fs = slice(cu * FCH, min(F_, (cu + 1) * FCH))
                w = fs.stop - fs.start
                nc.scalar.copy(out=stage[:, fs], in_=ps_of.pop(cu)[:, 0:w])
                del nvisit[cu]
                # the final DMA is on the critical path (epilogue barrier):
                # ship the last two chunks together so the closing transfer
                # is 1.7KB/partition instead of an inefficient 640B one
                if cu == NCH - 2:
                    return
                if cu == NCH - 1:
                    fs = slice((NCH - 2) * FCH, F_)
                nc.sync.dma_start(out=out_d[:, fs], in_=stage[:, fs])

            for u, (c, b) in enumerate(units):
                fs = slice(c * FCH, min(F_, (c + 1) * FCH))
                w = fs.stop - fs.start
                qrs = []
                for j, q in enumerate(PAIR_BURSTS[b]):
                    p_loc = 3 * b + j        # chunk-local emission position
                    xo = p_loc * 128
                    xb = boot_sb if p_loc < 6 else xin_sb
                    if c == 0:
                        wb, ws = boot_sb, slice(768, 768 + w)
                    else:
                        wb, ws = wt_sb, fs
                    pc = pconv.tile([128, 1024], _f32, tag="pc")
                    nc.tensor.matmul(
                        out=pc[:, 0:w],
                        lhsT=xb[0:KC, xo:xo + 128],
                        rhs=wb[0:KC, ws], start=True, stop=True)
                    nc.tensor.matmul(
                        out=pc[:, 512:512 + w],
                        lhsT=xb[64:64 + KC, xo:xo + 128],
                        rhs=wb[64:64 + KC, ws],
                        start=True, stop=True)
                    r = rpool.tile([128, 1024], _f16, tag="r")
                    # relu split: 7 on vector, 7 + per-chunk cast on scalar;
                    # chunk 0 runs 6/8 (scalar has slack during pipeline ramp)
                    if (p_loc % 2 == 0 if c else p_loc % 2 == 1 and p_loc < 13):
                        nc.vector.tensor_scalar(out=r[:, 0:512 + w],
                                                in0=pc[:, 0:512 + w],
                                                scalar1=0.0, scalar2=None,
                                                op0=max_op)
                    else:
                        nc.scalar.activation(out=r[:, 0:512 + w],
                                             in_=pc[:, 0:512 + w],
                                             func=relu_fn)
                    qrs.append((q, r))
                pend.append((c, qrs))
                if casts:
                    do_cast(casts.pop(0))
                if u >= 3:
                    cu, qrs_u = pend.pop(0)
                    do_sums(cu, qrs_u)
                    if (u - 3) % NB == NB - 1:
                        casts.append(cu)
            for cu, qrs_u in pend:
                do_sums(cu, qrs_u)
                if casts:
                    do_cast(casts.pop(0))
            casts.append(NCH - 1)
            while casts:
                do_cast(casts.pop(0))

    nc.compile()
    return nc


def _get_program():
    global _cached_nc
    if _cached_nc is None:
        _cached_nc = _build_program()
    return _cached_nc


def _host_prep(seqs, weight, bias):
    s = np.asarray(seqs, np.float32).reshape(NM, L_, B_)[:, :, :A_]
    sw = sliding_window_view(s, K_, axis=1)          # (NM, 28, 20, 3)
    X = sw.transpose(3, 2, 0, 1).reshape(A_ * K_, NM, LOUT)
    X = np.concatenate([X, np.ones((1, NM, LOUT), np.float32)], axis=0)

    Wt = np.asarray(weight, np.float32).transpose(2, 1, 0).reshape(A_ * K_, F_)
    Wb = np.concatenate([Wt, np.asarray(bias, np.float32)[None, :]], axis=0)
    wt = np.zeros((128, F_), np.float32)
    wt[0:KC] = Wb
    wt[64:64 + KC] = Wb
    wt_f16 = wt.astype(np.float16)

    # G_t[j, m] = 1 iff position 128t+j belongs to output row m of col group t//7
    G = np.zeros((128, NT * 32), np.float16)
    for t in range(NT):
        nm_of_j = (128 * t + np.arange(128)) // LOUT
        G[np.arange(128), 32 * t + nm_of_j % 32] = 1.0

    in_maps = []
    for c in range(CORES):
        Xc = X[:, c * NMC:(c + 1) * NMC, :].reshape(KC, NT, 128)
        xin = np.zeros((128, NP, 128), np.float32)
        xin[0:KC] = Xc[:, [2 * q for q in PAIR_EMIT]]
        xin[64:64 + KC] = Xc[:, [2 * q + 1 for q in PAIR_EMIT]]
        xin = np.ascontiguousarray(
            xin.reshape(128, NP * 128)).astype(np.float16)
        boot = np.ascontiguousarray(
            np.concatenate([xin[:, 0:768], wt_f16[:, 0:FCH]], axis=1))
        in_maps.append({"xin": xin, "wt": wt_f16, "g": G, "boot": boot})
    return in_maps


def run_bass(seqs, weight, bias, trace=False):
    """Returns (out (32,32,8000) float32, exec_time_ns or None)."""
    nc = _get_program()
    in_maps = _host_prep(seqs, weight, bias)
    res = run_bass_kernel_spmd(nc, in_maps, list(range(CORES)), trace=trace)
    out = np.concatenate([res.results[c]["out"] for c in range(CORES)], axis=0)
    return out.reshape(N_, M_, F_).astype(np.float32), res.exec_time_ns


def kernel(seqs, weight, bias):
    out, _ = run_bass(seqs, weight, bias, trace=False)
    return out
